# revision 9
# baseline (speedup 1.0000x reference)
"""Multi-head attention (RoPE, causal) Trainium2 Bass kernel, 8-core SPMD.

Sharding: batch (2) x head-groups (4 heads/core). Per core:
  - QKV projections for its 4 heads (tensor-parallel over heads)
  - RoPE + causal attention for its 4 heads (2 head-pairs packed into
    128 partitions, causality exploited at 128-key tile granularity)
  - Per-query-chunk AllGather (4 x 256KB bf16) pipelined with attention
  - Output projection sharded by output dim (per-core Wo slice)

v2: bf16 datapath end-to-end (fp32 PSUM accumulation and fp32 output).
Scores for the two heads of a pair run as concurrent row-group-packed
matmuls (tile_position), softmax denominators ride the PV matmul as a
65th ones-column of V, normalization uses reciprocal_approx_fast, and
the AllGather is split per 512-token query chunk so only the last
chunk's collective sits on the critical path. The output projection
consumes the gathered chunks directly (Wo rows pre-permuted on host).
"""

import functools
import os

import numpy as np

os.environ.setdefault("MYCRO_LOCAL_CACHE", "1")

D_MODEL = 1024
NUM_HEADS = 16
D_K = 64
THETA = 10000.0
B = 2
S = 2048
N_CORES = 8
GROUPS = [[0, 1, 2, 3], [4, 5, 6, 7]]
HPC = 4            # heads per core
DIMS = HPC * D_K   # 256 head-dims per core
OC = D_MODEL // 4  # 256 output dims per core (final projection)
NKT = S // 128     # 16 key tiles
NQC = S // 512     # 4 query chunks


def _build_nc(debug=False, repeat=1, collective=True):
    from contextlib import ExitStack

    import concourse.tile as tile
    from concourse import bacc, mybir

    F32 = mybir.dt.float32
    BF = mybir.dt.bfloat16
    EXP = mybir.ActivationFunctionType.Exp

    nc = bacc.Bacc(
        "TRN2",
        target_bir_lowering=False,
        debug=False,
        enable_asserts=False,
        num_devices=N_CORES,
    )

    xT_d = nc.dram_tensor("xT", [D_MODEL, S], BF, kind="ExternalInput")
    wqT_d = nc.dram_tensor("wqT", [D_MODEL, DIMS], BF, kind="ExternalInput")
    wkT_d = nc.dram_tensor("wkT", [D_MODEL, DIMS], BF, kind="ExternalInput")
    wvT_d = nc.dram_tensor("wvT", [D_MODEL, DIMS], BF, kind="ExternalInput")
    woT_d = nc.dram_tensor("woT", [D_MODEL, OC], BF, kind="ExternalInput")
    cos_d = nc.dram_tensor("cosT", [128, S], BF, kind="ExternalInput")
    sin_d = nc.dram_tensor("sinT", [128, S], BF, kind="ExternalInput")
    prot_d = nc.dram_tensor("prot", [128, 128], BF, kind="ExternalInput")
    tri_d = nc.dram_tensor("tri2", [128, 256], BF, kind="ExternalInput")
    out_d = nc.dram_tensor("out", [S, OC], F32, kind="ExternalOutput")

    bounce_c = [
        nc.dram_tensor(f"bounce{c}", [128, 2, 512], BF) for c in range(NQC)
    ]
    ag_c = [
        nc.dram_tensor(f"ag_out{c}", [512, 2, 512], BF) for c in range(NQC)
    ]
    if debug:
        qdump = nc.dram_tensor("qdump", [128, 2, S], BF, kind="ExternalOutput")
        kdump = nc.dram_tensor("kdump", [128, 2, S], BF, kind="ExternalOutput")
        vdump = nc.dram_tensor("vdump", [128, NKT, 4, 65], BF, kind="ExternalOutput")
        aodump = nc.dram_tensor("aodump", [128, 2, S], BF, kind="ExternalOutput")
        agdump = nc.dram_tensor("agdump", [512, 2, 512], BF, kind="ExternalOutput")
        odump = nc.dram_tensor("odump", [65, 512], F32, kind="ExternalOutput")
        udump = nc.dram_tensor("udump", [128, 2, 512], BF, kind="ExternalOutput")

    with tile.TileContext(nc) as tc, ExitStack() as ctx:
        ctx.enter_context(nc.allow_low_precision(reason="2e-2 tolerance, bf16 ok"))
        const = ctx.enter_context(tc.tile_pool(name="const", bufs=1))

        wq_sb = const.tile([128, 8, DIMS], BF)
        wk_sb = const.tile([128, 8, DIMS], BF)
        wv_sb = const.tile([128, 8, DIMS], BF)
        cos_sb = const.tile([128, S], BF)
        sin_sb = const.tile([128, S], BF)
        prot_sb = const.tile([128, 128], BF)
        tri_sb = const.tile([128, 256], BF)
        wo_sb = const.tile([128, 8, OC], BF)

        wq_r = wqT_d.ap().rearrange("(k p) m -> p k m", p=128)
        wk_r = wkT_d.ap().rearrange("(k p) m -> p k m", p=128)
        wv_r = wvT_d.ap().rearrange("(k p) m -> p k m", p=128)
        for k in range(8):
            nc.sync.dma_start(wq_sb[:, k, :], wq_r[:, k, :])

        for rep in range(repeat):
          with tc.tile_pool(name=f"qkv{rep}", bufs=1) as qkvpool:
            qp_sb = qkvpool.tile([128, 2, S], BF, name=f"qp_{rep}")
            kp_sb = qkvpool.tile([128, 2, S], BF, name=f"kp_{rep}")
            v_sb = qkvpool.tile([128, NKT, 4, 65], BF, name=f"v_{rep}")
            ao_sb = qkvpool.tile([128, 2, S], BF, name=f"ao_{rep}")
            # ---------------- Phase 1: QKV projections + RoPE ----------------
            with (
                tc.tile_pool(name=f"xin{rep}", bufs=2) as xpool,
                tc.tile_pool(name=f"p1sb{rep}", bufs=3) as p1sb,
                tc.tile_pool(name=f"p1qk{rep}", bufs=3, space="PSUM") as qkps,
                tc.tile_pool(name=f"p1rot{rep}", bufs=2, space="PSUM") as rotps,
                tc.tile_pool(name=f"p1v{rep}", bufs=2, space="PSUM") as vps,
            ):
                xT_r = xT_d.ap().rearrange("(k p) (c w) -> p k c w", p=128, w=512)
                x_tiles = []
                for t in range(4):
                    xt = xpool.tile([128, 8, 512], BF, tag="x", name=f"x_{rep}_{t}")
                    x_tiles.append(xt)
                    if t > 0:
                        continue
                    for k in range(8):
                        nc.sync.dma_start(xt[:, k, :], xT_r[:, k, 0, :])
                    if rep == 0:
                        # remaining constants, in consumption order
                        for k in range(8):
                            nc.sync.dma_start(wk_sb[:, k, :], wk_r[:, k, :])
                        nc.sync.dma_start(prot_sb[:], prot_d[:])
                        nc.sync.dma_start(cos_sb[:], cos_d[:])
                        nc.sync.dma_start(sin_sb[:], sin_d[:])
                        for k in range(8):
                            nc.sync.dma_start(wv_sb[:, k, :], wv_r[:, k, :])
                        nc.sync.dma_start(tri_sb[:], tri_d[:])
                        nc.sync.dma_start(
                            wo_sb[:], woT_d.ap().rearrange("(k p) m -> p k m", p=128)
                        )
                    nc.gpsimd.memset(v_sb[:, :, :, 64:65], 1.0)
                for t in range(4):  # 512-token chunks
                    x_sb = x_tiles[t]
                    if t > 0:
                        for k in range(8):
                            nc.sync.dma_start(x_sb[:, k, :], xT_r[:, k, t, :])

                    # Q/K projections ([dims, tok] layout) + RoPE
                    for w_sb, dst in ((wq_sb, qp_sb), (wk_sb, kp_sb)):
                        for m in range(2):  # head-pair = partition tile of dims
                            ps = qkps.tile([128, 512], F32, tag="qk")
                            for k in range(8):
                                nc.tensor.matmul(
                                    ps[:],
                                    w_sb[:, k, 128 * m : 128 * m + 128],
                                    x_sb[:, k, :],
                                    start=(k == 0),
                                    stop=(k == 7),
                                )
                            q_sb = p1sb.tile([128, 512], BF, tag="qsb")
                            nc.vector.tensor_copy(q_sb[:], ps[:])
                            rps = rotps.tile([128, 512], F32, tag="rot")
                            nc.tensor.matmul(rps[:], prot_sb[:], q_sb[:], start=True, stop=True)
                            s1 = p1sb.tile([128, 512], BF, tag="s1")
                            nc.vector.tensor_mul(s1[:], rps[:], sin_sb[:, 512 * t : 512 * t + 512])
                            q2 = p1sb.tile([128, 512], BF, tag="q2")
                            nc.vector.tensor_mul(q2[:], q_sb[:], cos_sb[:, 512 * t : 512 * t + 512])
                            nc.gpsimd.tensor_add(
                                dst[:, m, 512 * t : 512 * t + 512], q2[:], s1[:]
                            )

                    # V projection ([tok, dims] layout)
                    for mt in range(4):
                        vp = vps.tile([128, DIMS], F32, tag="v")
                        for k in range(8):
                            nc.tensor.matmul(
                                vp[:],
                                x_sb[:, k, 128 * mt : 128 * mt + 128],
                                wv_sb[:, k, :],
                                start=(k == 0),
                                stop=(k == 7),
                            )
                        nc.vector.tensor_copy(
                            v_sb[:, 4 * t + mt, :, 0:64],
                            vp[:].rearrange("p (h d) -> p h d", d=64),
                        )

            # ---------------- Phase 2: causal attention ----------------
            with (
                tc.tile_pool(name=f"usb{rep}", bufs=4) as upool,
                tc.tile_pool(name=f"fin{rep}", bufs=3) as fpool,
                tc.tile_pool(name=f"sps{rep}", bufs=2, space="PSUM") as spool,
                tc.tile_pool(name=f"o0ps{rep}", bufs=2, space="PSUM") as o0pool,
                tc.tile_pool(name=f"o1ps{rep}", bufs=2, space="PSUM") as o1pool,
            ):
                tri_v = tri_sb[:].rearrange("q (b w) -> q b w", b=2)
                from concourse import mybir as _mb
                for c in range(NQC):  # 512-wide query chunks
                    for p in range(2):  # head pairs
                        o65 = [
                            o0pool.tile([65, 512], F32, tag="o0", name=f"o65a_{rep}_{p}_{c}"),
                            o1pool.tile([65, 512], F32, tag="o1", name=f"o65b_{rep}_{p}_{c}"),
                        ]
                        nk = 4 * c + 4
                        for k in range(nk):
                            jd = k - 4 * c  # >= 0 on the diagonal band
                            lo = 128 * jd if jd >= 0 else 0
                            sp = spool.tile([128, 2, 512], F32, tag="s")
                            for h in (0, 1):
                                nc.tensor.matmul(
                                    sp[:, h, lo:512],
                                    kp_sb[64 * h : 64 * h + 64, p, 128 * k : 128 * k + 128],
                                    qp_sb[64 * h : 64 * h + 64, p, 512 * c + lo : 512 * c + 512],
                                    start=True,
                                    stop=True,
                                )
                            u = upool.tile([128, 2, 512], BF, tag="u")
                            nc.scalar.activation(
                                u[:, :, lo:512], sp[:, :, lo:512], EXP, scale=0.125
                            )
                            if jd >= 0:
                                nc.vector.tensor_mul(
                                    u[:, :, lo : lo + 128], u[:, :, lo : lo + 128], tri_v
                                )
                            if debug and p == 0 and c == 0 and k == 0:
                                nc.sync.dma_start(udump[:], u[:])
                            for h in (0, 1):
                                nc.tensor.matmul(
                                    o65[h][:, lo:512],
                                    v_sb[:, k, 2 * p + h, :],
                                    u[:, h, lo:512],
                                    start=(k == 0),
                                    stop=(k == nk - 1),
                                )
                        if debug and p == 0 and c == 0:
                            osb = fpool.tile([65, 512], F32, name=f"osb_{rep}")
                            nc.vector.tensor_copy(osb[:], o65[0][:])
                            nc.sync.dma_start(odump[:], osb[:])
                        for h in (0, 1):
                            den = fpool.tile([1, 512], F32, tag="r", name=f"den_{rep}_{p}_{c}_{h}")
                            nc.vector.tensor_copy(den[:], o65[h][64:65, :])
                            bcast = fpool.tile([64, 512], F32, tag="b", name=f"bc_{rep}_{p}_{c}_{h}")
                            nc.gpsimd.partition_broadcast(bcast[:], den[:])
                            rcpw = fpool.tile([64, 512], F32, tag="rw", name=f"rw_{rep}_{p}_{c}_{h}")
                            nc.vector.reciprocal_approx_fast(rcpw[:], bcast[:])
                            nc.vector.tensor_mul(
                                ao_sb[64 * h : 64 * h + 64, p, 512 * c : 512 * c + 512],
                                o65[h][0:64, :],
                                rcpw[:],
                            )
                    # per-chunk AllGather, overlapped with later chunks
                    nc.sync.dma_start(bounce_c[c][:], ao_sb[:, :, 512 * c : 512 * c + 512])
                    if collective:
                        nc.gpsimd.collective_compute(
                            "AllGather",
                            _mb.AluOpType.bypass,
                            ins=[bounce_c[c][:]],
                            outs=[ag_c[c][:]],
                            replica_groups=GROUPS,
                        )
                    else:
                        nc.sync.dma_start(ag_c[c][0:128, :, :], bounce_c[c][:])

            # ---------------- Phase 3: output projection ----------------
            if debug:
                nc.sync.dma_start(qdump[:], qp_sb[:])
                nc.sync.dma_start(kdump[:], kp_sb[:])
                nc.sync.dma_start(vdump[:], v_sb[:])
                nc.sync.dma_start(aodump[:], ao_sb[:])
                nc.sync.dma_start(agdump[:], ag_c[0].ap())
            with (
                tc.tile_pool(name=f"ph3{rep}", bufs=2) as agpool,
                tc.tile_pool(name=f"ph3o{rep}", bufs=2) as outpool,
                tc.tile_pool(name=f"ph3p{rep}", bufs=6, space="PSUM") as pps,
            ):
                out_r = out_d.ap().rearrange("(n p) o -> p n o", p=128)
                for c in range(NQC):
                    agt = agpool.tile([128, 4, 2, 512], BF, tag="ag", name=f"ag_{rep}_{c}")
                    nc.sync.dma_start(
                        agt[:], ag_c[c].ap().rearrange("(r q) pp w -> q r pp w", q=128)
                    )
                    for tt in range(4):
                        po = pps.tile([128, OC], F32, tag="po")
                        idx = 0
                        for r in range(4):
                            for pp in range(2):
                                nc.tensor.matmul(
                                    po[:],
                                    agt[:, r, pp, 128 * tt : 128 * tt + 128],
                                    wo_sb[:, 2 * r + pp, :],
                                    start=(idx == 0),
                                    stop=(idx == 7),
                                )
                                idx += 1
                        ot = outpool.tile([128, OC], F32, tag="ot")
                        nc.vector.tensor_copy(ot[:], po[:])
                        nc.sync.dma_start(out_r[:, 4 * c + tt, :], ot[:])

    nc.compile()
    return nc


@functools.lru_cache(maxsize=4)
def _get_nc(repeat=1):
    return _build_nc(repeat=repeat)


def _host_inputs(x, Wq, Wk, Wv, Wo):
    """Host-side prep: per-core slices, transposes, RoPE tables (bf16)."""
    import ml_dtypes

    f32 = np.float32
    bf16 = ml_dtypes.bfloat16
    inv_freq = (1.0 / (THETA ** (np.arange(0, D_K, 2, dtype=f32) / D_K))).astype(f32)
    t = np.arange(S, dtype=f32)
    freqs = t[:, None] * inv_freq[None, :]  # [S, 32]
    cos = np.cos(freqs).astype(f32)
    sin = np.sin(freqs).astype(f32)
    # [128, S] tables in [dim, tok] layout, periodic per 64 dims
    didx = (np.arange(128) % 64) // 2
    cosT = np.ascontiguousarray(cos[:, didx].T).astype(bf16)
    sinT = np.ascontiguousarray(sin[:, didx].T).astype(bf16)

    prot = np.zeros((128, 128), dtype=f32)
    g = np.arange(64)
    prot[2 * g + 1, 2 * g] = -1.0
    prot[2 * g, 2 * g + 1] = 1.0
    prot = prot.astype(bf16)

    tri = (np.arange(128)[None, :] >= np.arange(128)[:, None]).astype(f32)
    tri2 = np.ascontiguousarray(np.tile(tri, (1, 2))).astype(bf16)

    xT = [np.ascontiguousarray(x[b].T).astype(bf16) for b in range(B)]
    # Wo.T rows permuted to the AllGather row order (rank r, pair p, q):
    # row r*256 + p*128 + q  <-  Wo column (4r + 2p + q//64)*64 + q%64
    woT_n = Wo.T
    ridx = np.arange(D_MODEL)
    r_, rem = ridx // 256, ridx % 256
    p_, q_ = rem // 128, rem % 128
    src = (4 * r_ + 2 * p_ + q_ // 64) * 64 + q_ % 64
    woT = np.ascontiguousarray(woT_n[src, :])

    in_maps = []
    for c in range(N_CORES):
        b, j = c // 4, c % 4
        in_maps.append(
            {
                "xT": xT[b],
                "wqT": np.ascontiguousarray(Wq[DIMS * j : DIMS * (j + 1), :].T).astype(bf16),
                "wkT": np.ascontiguousarray(Wk[DIMS * j : DIMS * (j + 1), :].T).astype(bf16),
                "wvT": np.ascontiguousarray(Wv[DIMS * j : DIMS * (j + 1), :].T).astype(bf16),
                "woT": np.ascontiguousarray(woT[:, OC * j : OC * (j + 1)]).astype(bf16),
                "cosT": cosT,
                "sinT": sinT,
                "prot": prot,
                "tri2": tri2,
            }
        )
    return in_maps


@functools.lru_cache(maxsize=4)
def _get_exec(repeat=1):
    """Build the bass program once and return a persistent jitted callable.

    Mirrors concourse.bass2jax.run_bass_via_pjrt, but caches the jitted
    shard_map so repeated kernel() calls don't re-trace/re-compile.
    """
    import jax
    from jax.sharding import Mesh, PartitionSpec
    from jax.experimental.shard_map import shard_map

    from concourse import bass2jax, mybir

    nc = _get_nc(repeat)
    bass2jax.install_neuronx_cc_hook()

    partition_name = nc.partition_id_tensor.name if nc.partition_id_tensor else None
    in_names, out_names, out_avals = [], [], []
    for alloc in nc.m.functions[0].allocations:
        if not isinstance(alloc, mybir.MemoryLocationSet):
            continue
        name = alloc.memorylocations[0].name
        if alloc.kind == "ExternalInput":
            if name != partition_name:
                in_names.append(name)
        elif alloc.kind == "ExternalOutput":
            out_names.append(name)
            out_avals.append(
                jax.core.ShapedArray(
                    tuple(alloc.tensor_shape), mybir.dt.np(alloc.dtype)
                )
            )
    n_params = len(in_names)
    all_names = in_names + out_names
    if partition_name is not None:
        all_names = all_names + [partition_name]

    def _body(*args):
        operands = list(args)
        if partition_name is not None:
            operands.append(bass2jax.partition_id_tensor())
        return tuple(
            bass2jax._bass_exec_p.bind(
                *operands,
                out_avals=tuple(out_avals),
                in_names=tuple(all_names),
                out_names=tuple(out_names),
                lowering_input_output_aliases=(),
                sim_require_finite=True,
                sim_require_nnan=True,
                nc=nc,
            )
        )

    devices = jax.devices()[:N_CORES]
    mesh = Mesh(np.asarray(devices), ("core",))
    n_outs = len(out_names)
    donate = tuple(range(n_params, n_params + n_outs))
    sharded = jax.jit(
        shard_map(
            _body,
            mesh=mesh,
            in_specs=(PartitionSpec("core"),) * (n_params + n_outs),
            out_specs=(PartitionSpec("core"),) * n_outs,
            check_rep=False,
        ),
        donate_argnums=donate,
        keep_unused=True,
    )
    zero_protos = [
        (tuple((N_CORES * a.shape[0], *a.shape[1:])), a.dtype) for a in out_avals
    ]
    out_shapes = [tuple(a.shape) for a in out_avals]
    return sharded, in_names, out_names, n_params, zero_protos, out_shapes


def _concat_inputs(in_maps):
    _, in_names, _, _, _, _ = _get_exec()
    return [
        np.concatenate([np.asarray(in_maps[c][n]) for c in range(N_CORES)], axis=0)
        for n in in_names
    ]


def _exec(concat_in, as_numpy=True, repeat=1):
    sharded, _, out_names, _, zero_protos, out_shapes = _get_exec(repeat)
    zeros = [np.zeros(shape, dt) for shape, dt in zero_protos]
    out_arrs = sharded(*concat_in, *zeros)
    if not as_numpy:
        return out_arrs
    return [
        {
            n: np.asarray(out_arrs[i]).reshape(N_CORES, *out_shapes[i])[c]
            for i, n in enumerate(out_names)
        }
        for c in range(N_CORES)
    ]


def _run(in_maps):
    return _exec(_concat_inputs(in_maps))


def kernel(x, Wq, Wk, Wv, Wo):
    in_maps = _host_inputs(
        np.asarray(x), np.asarray(Wq), np.asarray(Wk), np.asarray(Wv), np.asarray(Wo)
    )
    results = _run(in_maps)
    out = np.empty((B, S, D_MODEL), dtype=np.float32)
    for c in range(N_CORES):
        b, j = c // 4, c % 4
        out[b, :, OC * j : OC * (j + 1)] = results[c]["out"]
    return out
